# revision 29
# baseline (speedup 1.0000x reference)
"""AttentionLePE distributed Trainium2 kernel.

Strategy: pure data-parallel over batch (8 batch elements -> 8 NeuronCores,
no collectives). Per core, the full attention block runs with:
  - bf16 matmuls on TensorE (qkv, s^T = k q^T with 4-head row-packing,
    p@v + all-ones denominator matmuls with 4-head col-packing, proj)
  - softmax exp on ScalarE straight out of PSUM (no max-subtraction: logits
    are ~N(0,1) so exp is safe in f32)
  - LePE 5x5 depthwise conv split across engines: the 3x3 inner taps as
    accumulating diagonal matmuls on TensorE with spatially shifted access
    patterns (zero-pad handled by trimming), the 16 outer-ring taps as fused
    affine MACs (affine_then_add) on VectorE
  - normalization via all-ones matmul row sums (replicated to each head's 32
    output rows) -> fast reciprocal on VectorE straight from PSUM -> fused
    multiply during PSUM evacuation

The whole kernel is software-pipelined at emission time as one flat 64-step
sweep sequence: each engine's in-order stream gets p@v of step m-1 plus
deadline-scheduled filler (v/qk tiles, LePE taps, early proj halves) between
s^T(m) and s^T(m+1), so TensorE stays busy while ScalarE runs exp(m) and
exp never stalls at sweep boundaries.

Host side pre-transposes x and all weights so no on-device transposes are
needed, and folds b_lepe through w_proj into an effective bias.
"""

import numpy as np
import ml_dtypes

B, Hs, Ws, C = 8, 32, 32, 512
N = Hs * Ws          # 1024 tokens
HEADS = 16
HD = C // HEADS      # 32
KS = 5
SCALE = float(HD) ** -0.5
NCORES = 8

_BF16 = ml_dtypes.bfloat16

LAST_EXEC_TIME_NS = None
LAST_RESULTS = None


def _build_graph():
    import os as _os
    LOOP = int(_os.environ.get("ATTN_LEPE_LOOP", "1"))
    DBG = _os.environ.get("ATTN_LEPE_DEBUG", "") == "1"
    import concourse.bacc as bacc
    import concourse.mybir as mybir
    import concourse.tile as tile

    dt = mybir.dt
    AF = mybir.ActivationFunctionType

    nc = bacc.Bacc(
        "TRN2",
        target_bir_lowering=False,
        debug=False,
        enable_asserts=False,
        num_devices=NCORES,
    )

    xT_d = nc.dram_tensor("xT", [C, N], dt.bfloat16, kind="ExternalInput")
    wqkT_d = nc.dram_tensor("wqkT", [C, 2 * C], dt.bfloat16, kind="ExternalInput")
    wvT_d = nc.dram_tensor("wvT", [C, C], dt.bfloat16, kind="ExternalInput")
    wpT_d = nc.dram_tensor("wpT", [C, C], dt.bfloat16, kind="ExternalInput")
    ones_d = nc.dram_tensor("ones", [128, 32], dt.bfloat16, kind="ExternalInput")
    # lepe_d[p, (pi*4+g)*128 + q] = (p==q) * w_lepe[128*g+p, inner tap pi]
    lepe_d = nc.dram_tensor("lepe", [128, 9 * 4 * 128], dt.bfloat16,
                            kind="ExternalInput")
    lepec_d = nc.dram_tensor("lepec", [128, KS * KS * 4], dt.float32,
                             kind="ExternalInput")
    beff_d = nc.dram_tensor("beff", [128, 4], dt.float32, kind="ExternalInput")
    out_d = nc.dram_tensor("out", [C, N], dt.float32, kind="ExternalOutput")
    if DBG:
        dbg_pT = nc.dram_tensor("dbg_pT", [128, 2048], dt.bfloat16,
                                kind="ExternalOutput")
        dbg_ao0 = nc.dram_tensor("dbg_ao0", [128, N], dt.bfloat16,
                                 kind="ExternalOutput")

    NT = N // 128   # 8 token tiles
    CT = C // 128   # 4 channel tiles
    NC2 = N // 512  # 2 n-chunks

    taps = [(0, 0)] + [
        (dh, dw) for dh in range(-2, 3) for dw in range(-2, 3) if (dh, dw) != (0, 0)
    ]

    with tile.TileContext(nc) as tc:
        with (
            tc.tile_pool(name="persist", bufs=1) as persist,
            tc.tile_pool(name="pT", bufs=4) as pT_pool,
            tc.tile_pool(name="dr", bufs=3) as dr_pool,
            tc.tile_pool(name="ps_big", bufs=1, space="PSUM") as ps_big,
            tc.tile_pool(name="ps_small", bufs=2, space="PSUM") as ps_small,
            tc.tile_pool(name="ps_den", bufs=1, space="PSUM") as ps_den,
            tc.tile_pool(name="ps_lepe", bufs=1, space="PSUM") as ps_lepe,
        ):
            # ---- persistent SBUF loads ----
            xT = []
            for g in range(CT):
                t = persist.tile([128, N], dt.bfloat16, tag=f"xT{g}", name=f"xT{g}")
                nc.sync.dma_start(t[:], xT_d[g * 128:(g + 1) * 128, :])
                xT.append(t)
            wqkT = []
            for g in range(CT):
                t = persist.tile([128, 2 * C], dt.bfloat16, tag=f"wqkT{g}",
                                 name=f"wqkT{g}")
                nc.sync.dma_start(t[:], wqkT_d[g * 128:(g + 1) * 128, :])
                wqkT.append(t)
            wvT = []
            for g in range(CT):
                t = persist.tile([128, C], dt.bfloat16, tag=f"wvT{g}", name=f"wvT{g}")
                nc.sync.dma_start(t[:], wvT_d[g * 128:(g + 1) * 128, :])
                wvT.append(t)
            ones_sb = persist.tile([128, 32], dt.bfloat16, tag="ones", name="ones_sb")
            nc.sync.dma_start(ones_sb[:], ones_d[:, :])
            # non-critical loads (LePE table, proj weights) are deferred past
            # the head kickoff so the x/w_qk/w_v preload gets full DMA
            # bandwidth
            wpT = [persist.tile([128, C], dt.bfloat16, tag=f"wpT{g}",
                                name=f"wpT{g}") for g in range(CT)]
            lepe_w = persist.tile([128, 9 * 4 * 128], dt.bfloat16, tag="lepe",
                                  name="lepe_w")
            beff_sb = persist.tile([128, 4], dt.float32, tag="beff", name="beff_sb")
            lepec_sb = persist.tile([128, KS * KS * 4], dt.float32, tag="lepec",
                                    name="lepec_sb")

            def load_noncritical():
                nc.sync.dma_start(lepe_w[:], lepe_d[:, :])
                nc.sync.dma_start(lepec_sb[:], lepec_d[:, :])
                for g in range(CT):
                    nc.sync.dma_start(wpT[g][:], wpT_d[g * 128:(g + 1) * 128, :])
                nc.sync.dma_start(beff_sb[:], beff_d[:, :])

            for _it in range(LOOP):
                # ---------- tiles ----------
                v_sb = [persist.tile([128, 512], dt.bfloat16, tag=f"v{m}",
                                     name=f"v{m}") for m in range(NT)]
                qk_sb = [
                    persist.tile([128, N], dt.bfloat16, tag=f"qk{f}", name=f"qk{f}")
                    for f in range(8)
                ]
                aoT = [persist.tile([128, N], dt.bfloat16, tag=f"aoT{g}",
                                    name=f"aoT{g}") for g in range(4)]
                yT_sb = [persist.tile([128, N], dt.float32, tag=f"yT{co}",
                                      name=f"yT{co}") for co in range(CT)]
                x3 = [xT[g][:].rearrange("p (h w) -> p h w", w=Ws)
                      for g in range(CT)]

                def emit_v(m):
                    v_ps = ps_small.tile([128, 512], dt.float32, tag="sm",
                                         name=f"vps{m}")
                    for c in range(CT):
                        nc.tensor.matmul(
                            out=v_ps[:],
                            lhsT=xT[c][:, m * 128:(m + 1) * 128],
                            rhs=wvT[c][:],
                            start=(c == 0), stop=(c == CT - 1),
                        )
                    nc.vector.tensor_copy(v_sb[m][:], v_ps[:])

                def emit_qk(f, nc2):
                    qk_ps = ps_small.tile([128, 512], dt.float32, tag="sm",
                                          name=f"qkps{f}_{nc2}")
                    for c in range(CT):
                        nc.tensor.matmul(
                            out=qk_ps[:],
                            lhsT=wqkT[c][:, f * 128:(f + 1) * 128],
                            rhs=xT[c][:, nc2 * 512:(nc2 + 1) * 512],
                            start=(c == 0), stop=(c == CT - 1),
                        )
                    nc.vector.tensor_copy(
                        qk_sb[f][:, nc2 * 512:(nc2 + 1) * 512], qk_ps[:])

                def emit_proj(co, nc2):
                    ncs = slice(nc2 * 512, (nc2 + 1) * 512)
                    y_ps = ps_small.tile([128, 512], dt.float32, tag="sm",
                                         name=f"yps{co}_{nc2}")
                    for c in range(CT):
                        nc.tensor.matmul(
                            out=y_ps[:],
                            lhsT=wpT[c][:, co * 128:(co + 1) * 128],
                            rhs=aoT[c][:, ncs],
                            start=(c == 0), stop=(c == CT - 1),
                        )
                    nc.vector.tensor_scalar_add(
                        yT_sb[co][:, ncs], y_ps[:], beff_sb[:, co:co + 1])
                    # ship each output half as soon as its proj is done
                    nc.sync.dma_start(out_d[co * 128:(co + 1) * 128, ncs],
                                      yT_sb[co][:, ncs])

                # 3x3 inner taps on TensorE (diag matmuls), 16 outer-ring
                # taps on VectorE (fused affine MAC)
                pe_taps = [(dh, dw) for (dh, dw) in taps
                           if abs(dh) <= 1 and abs(dw) <= 1]
                dve_taps = [t for t in taps if t not in pe_taps]

                def lepe_mms(g, hb, lp3):
                    mms = []
                    for pi, (dh, dw) in enumerate(pe_taps):
                        r0, r1 = max(0, -dh), Hs - max(0, dh)
                        w0, w1 = max(0, -dw), Ws - max(0, dw)
                        hr0, hr1 = max(r0, hb * 16), min(r1, hb * 16 + 16)
                        if hr0 >= hr1:
                            continue
                        diag = lepe_w[:, (pi * 4 + g) * 128:(pi * 4 + g + 1) * 128]

                        def mm(pi=pi, hr0=hr0, hr1=hr1, w0=w0, w1=w1,
                               dh=dh, dw=dw, diag=diag, lp3=lp3, g=g, hb=hb):
                            nc.tensor.matmul(
                                out=lp3[:, hr0 - hb * 16:hr1 - hb * 16, w0:w1],
                                lhsT=diag,
                                rhs=x3[g][:, hr0 + dh:hr1 + dh, w0 + dw:w1 + dw],
                                start=(pi == 0), stop=(pi == len(pe_taps) - 1),
                                skip_group_check=True,
                            )
                        mms.append(mm)
                    return mms

                def lepe_dve_units(g, hb, acc):
                    acc3 = acc[:].rearrange("p (h w) -> p h w", w=Ws)
                    units = []
                    for dh, dw in dve_taps:
                        ti = taps.index((dh, dw))
                        r0, r1 = max(0, -dh), Hs - max(0, dh)
                        w0, w1 = max(0, -dw), Ws - max(0, dw)
                        hr0, hr1 = max(r0, hb * 16), min(r1, hb * 16 + 16)
                        if hr0 >= hr1:
                            continue

                        def u(ti=ti, hr0=hr0, hr1=hr1, w0=w0, w1=w1,
                              dh=dh, dw=dw, acc3=acc3, g=g, hb=hb):
                            dst = acc3[:, hr0 - hb * 16:hr1 - hb * 16, w0:w1]
                            nc.vector.affine_then_add(
                                out=dst,
                                in0=x3[g][:, hr0 + dh:hr1 + dh, w0 + dw:w1 + dw],
                                in1=dst,
                                scale=lepec_sb[:, ti * 4 + g:ti * 4 + g + 1],
                                bias=0.0,
                            )
                        units.append(u)
                    return units

                # ---------- head: minimum to start sweep (nc2=0, g=0) ----------
                emit_qk(4, 0)
                emit_qk(0, 0)
                if _it == 0:
                    load_noncritical()

                # filler units with emission deadlines (global step index)
                fillers = [(1, lambda: emit_v(0)), (2, lambda: emit_v(1))]
                for m in range(2, NT):
                    fillers.append((m + 1, lambda m=m: emit_v(m)))
                qk_sched = [((4, 1), 4), ((5, 0), 8), ((5, 1), 8), ((1, 0), 8),
                            ((6, 0), 16), ((6, 1), 16), ((2, 0), 16),
                            ((7, 0), 24), ((7, 1), 24), ((3, 0), 24),
                            ((0, 1), 32), ((1, 1), 40), ((2, 1), 48),
                            ((3, 1), 56)]
                for (f, nc2), dl in qk_sched:
                    fillers.append((dl, lambda f=f, nc2=nc2: emit_qk(f, nc2)))
                fillers.sort(key=lambda x: x[0])
                late = [(co, 0) for co in range(CT)]  # proj nc0 halves

                # ---------- 64-step flat pipeline over sweeps (nc2, g) ----------
                sweeps = [(nc2, g) for nc2 in range(NC2) for g in range(4)]
                steps = [(nc2, g, m) for (nc2, g) in sweeps for m in range(NT)]
                state = {}

                def sweep_tiles(nc2, g):
                    out_ps = ps_small.tile([128, 512], dt.float32, tag="sm",
                                           name=f"outps{g}_{nc2}")
                    den_ps = ps_den.tile([128, 512], dt.float32, tag="den",
                                         name=f"denps{g}_{nc2}")
                    lp_ps = ps_lepe.tile([128, 512], dt.float32, tag="lp",
                                         name=f"lp{g}_{nc2}")
                    lp3 = lp_ps[:].rearrange("p (h w) -> p h w", w=Ws)
                    acc = dr_pool.tile([128, 512], dt.bfloat16, tag="dveacc",
                                       name=f"acc{g}_{nc2}")
                    nc.gpsimd.memset(acc[:], 0.0)
                    return dict(out_ps=out_ps, den_ps=den_ps, lp_ps=lp_ps,
                                acc=acc, lepe=lepe_mms(g, nc2, lp3),
                                dve=lepe_dve_units(g, nc2, acc))

                def emit_sT(nc2, g, m):
                    ncs = slice(nc2 * 512, (nc2 + 1) * 512)
                    q_t, k_t = qk_sb[g], qk_sb[4 + g]
                    sT_ps = ps_big.tile([128, 2048], dt.float32, tag="big",
                                        name=f"sT{g}_{nc2}_{m}")
                    for j in range(4):
                        nc.tensor.matmul(
                            out=sT_ps[:, j * 512:(j + 1) * 512],
                            lhsT=k_t[j * 32:(j + 1) * 32, m * 128:(m + 1) * 128],
                            rhs=q_t[j * 32:(j + 1) * 32, ncs],
                            start=True, stop=True,
                            tile_position=(j * 32, 0),
                        )
                    pT = pT_pool.tile([128, 2048], dt.bfloat16, tag="pT",
                                      name=f"pT{g}_{nc2}_{m}")
                    nc.scalar.activation(pT[:], sT_ps[:], AF.Exp, scale=SCALE)
                    if DBG and _it == 0 and g == 0 and nc2 == 0 and m == 0:
                        nc.sync.dma_start(dbg_pT[:, :], pT[:])
                    return pT

                def emit_pv(nc2, g, m, pT):
                    st = state[(nc2, g)]
                    for j in range(4):
                        h = 4 * g + j
                        nc.tensor.matmul(
                            out=st["out_ps"][j * 32:(j + 1) * 32, :],
                            lhsT=v_sb[m][:, h * 32:(h + 1) * 32],
                            rhs=pT[:, j * 512:(j + 1) * 512],
                            start=(m == 0), stop=(m == NT - 1),
                            tile_position=(0, j * 32),
                            skip_group_check=True,
                        )
                        nc.tensor.matmul(
                            out=st["den_ps"][j * 32:(j + 1) * 32, :],
                            lhsT=ones_sb[:, 0:32],
                            rhs=pT[:, j * 512:(j + 1) * 512],
                            start=(m == 0), stop=(m == NT - 1),
                            tile_position=(0, j * 32),
                            skip_group_check=True,
                        )

                def emit_epilogue(nc2, g):
                    st = state.pop((nc2, g))
                    ncs = slice(nc2 * 512, (nc2 + 1) * 512)
                    drec = dr_pool.tile([128, 512], dt.float32, tag="drec",
                                        name="drec")
                    nc.vector.reciprocal_approx_fast(out=drec[:],
                                                     in_=st["den_ps"][:])
                    tmp = dr_pool.tile([128, 512], dt.float32, tag="ntmp",
                                       name="ntmp")
                    nc.vector.tensor_mul(tmp[:], st["out_ps"][:], drec[:])
                    nc.vector.tensor_add(
                        aoT[g][:, ncs], tmp[:], aoT[g][:, ncs])
                    if DBG and _it == 0 and g == 0 and nc2 == 1:
                        nc.sync.dma_start(dbg_ao0[:, :], aoT[g][:])

                prev = None       # (nc2, g, m, pT)
                for i, (nc2, g, m) in enumerate(steps):
                    while fillers and fillers[0][0] <= i:
                        fillers.pop(0)[1]()
                    if m == 0:
                        state[(nc2, g)] = sweep_tiles(nc2, g)
                    pT = emit_sT(nc2, g, m)
                    if prev is not None:
                        pnc2, pg, pm, ppT = prev
                        emit_pv(pnc2, pg, pm, ppT)
                        if pm == NT - 1:
                            emit_epilogue(pnc2, pg)
                    st = state[(nc2, g)]
                    # hold the sweep's first LePE ops one step so they don't
                    # stall on the previous sweep's epilogue chain
                    nmm = 0 if m == 0 else (1 if m < NT - 1 else len(st["lepe"]))
                    for _ in range(min(nmm, len(st["lepe"]))):
                        st["lepe"].pop(0)()
                    ndve = 0 if m == 0 else (3 if m < NT - 1 else len(st["dve"]))
                    for _ in range(min(ndve, len(st["dve"]))):
                        st["dve"].pop(0)()
                    if m == NT - 1:
                        # pre-merge LePE (PE psum + DVE acc) into aoT now,
                        # independent of exp(m)/p@v(m); the post-p@v epilogue
                        # then only needs recip -> mul -> one add
                        ncs_s = slice(nc2 * 512, (nc2 + 1) * 512)
                        nc.vector.tensor_add(
                            aoT[g][:, ncs_s], st["lp_ps"][:], st["acc"][:])
                    if late and i >= 34:
                        emit_proj(*late.pop(0))
                    elif fillers and (i % 2 == 1 or i < 8):
                        fillers.pop(0)[1]()
                    prev = (nc2, g, m, pT)

                # tail
                pnc2, pg, pm, ppT = prev
                emit_pv(pnc2, pg, pm, ppT)
                emit_epilogue(pnc2, pg)
                while late:
                    emit_proj(*late.pop(0))
                for co in range(CT):
                    emit_proj(co, 1)

    nc.finalize()
    return nc


_GRAPH = None


def kernel(x, w_qkv, w_proj, b_proj, w_lepe, b_lepe, _trace=False):
    global _GRAPH, LAST_EXEC_TIME_NS, LAST_RESULTS
    from concourse.bass_utils import run_bass_kernel_spmd

    x = np.asarray(x, dtype=np.float32)
    w_qkv = np.asarray(w_qkv, dtype=np.float32)
    w_proj = np.asarray(w_proj, dtype=np.float32)
    b_proj = np.asarray(b_proj, dtype=np.float32)
    w_lepe = np.asarray(w_lepe, dtype=np.float32)
    b_lepe = np.asarray(b_lepe, dtype=np.float32)

    wqkT = np.ascontiguousarray(w_qkv[:2 * C, :].T).astype(_BF16)   # [C, 2C]
    wvT = np.ascontiguousarray(w_qkv[2 * C:, :].T).astype(_BF16)    # [C, C]
    wpT = np.ascontiguousarray(w_proj.T).astype(_BF16)              # [C, C]
    beff = (w_proj @ b_lepe + b_proj).astype(np.float32)            # [C]
    beff_t = np.ascontiguousarray(beff.reshape(4, 128).T)           # [128, 4]

    taps = [(0, 0)] + [
        (dh, dw) for dh in range(-2, 3) for dw in range(-2, 3) if (dh, dw) != (0, 0)
    ]
    wl = w_lepe.reshape(C, KS, KS)  # tap (dh,dw) -> kernel[dh+2, dw+2]
    pe_taps = [(dh, dw) for (dh, dw) in taps if abs(dh) <= 1 and abs(dw) <= 1]
    lepe_flat = np.zeros((128, 9 * 4 * 128), dtype=_BF16)
    for pi, (dh, dw) in enumerate(pe_taps):
        for g in range(4):
            col0 = (pi * 4 + g) * 128
            wcol = wl[g * 128:(g + 1) * 128, dh + 2, dw + 2].astype(_BF16)
            lepe_flat[np.arange(128), col0 + np.arange(128)] = wcol
    ones128 = np.ones((128, 32), dtype=_BF16)
    lepe_col = np.zeros((128, KS * KS * 4), dtype=np.float32)
    for ti, (dh, dw) in enumerate(taps):
        for g in range(4):
            lepe_col[:, ti * 4 + g] = wl[g * 128:(g + 1) * 128, dh + 2, dw + 2]

    in_maps = []
    for b in range(NCORES):
        xT = np.ascontiguousarray(x[b].reshape(N, C).T).astype(_BF16)  # [C, N]
        in_maps.append({
            "xT": xT,
            "wqkT": wqkT,
            "wvT": wvT,
            "wpT": wpT,
            "ones": ones128,
            "lepe": lepe_flat,
            "lepec": lepe_col,
            "beff": beff_t,
        })

    if _GRAPH is None:
        _GRAPH = _build_graph()

    res = run_bass_kernel_spmd(_GRAPH, in_maps, list(range(NCORES)), trace=_trace)
    LAST_EXEC_TIME_NS = res.exec_time_ns
    LAST_RESULTS = res

    out = np.empty((B, Hs, Ws, C), dtype=np.float32)
    for b in range(NCORES):
        yT = np.asarray(res.results[b]["out"], dtype=np.float32)  # [C, N]
        out[b] = yT.T.reshape(Hs, Ws, C)
    return out


# revision 32
# speedup vs baseline: 1.0093x; 1.0093x over previous
"""AttentionLePE distributed Trainium2 kernel.

Strategy: pure data-parallel over batch (8 batch elements -> 8 NeuronCores,
no collectives). Per core, the full attention block runs with:
  - bf16 matmuls on TensorE (qkv, s^T = k q^T with 4-head row-packing,
    p@v + all-ones denominator matmuls with 4-head col-packing, proj)
  - softmax exp on ScalarE straight out of PSUM (no max-subtraction: logits
    are ~N(0,1) so exp is safe in f32)
  - LePE 5x5 depthwise conv split across engines: the 3x3 inner taps as
    accumulating diagonal matmuls on TensorE with spatially shifted access
    patterns (zero-pad handled by trimming), the 16 outer-ring taps as fused
    affine MACs (affine_then_add) on VectorE
  - normalization via all-ones matmul row sums (replicated to each head's 32
    output rows) -> fast reciprocal on VectorE straight from PSUM -> fused
    multiply during PSUM evacuation

The whole kernel is software-pipelined at emission time as one flat 64-step
sweep sequence: each engine's in-order stream gets p@v of step m-1 plus
deadline-scheduled filler (v/qk tiles, LePE taps, early proj halves) between
s^T(m) and s^T(m+1), so TensorE stays busy while ScalarE runs exp(m) and
exp never stalls at sweep boundaries.

Host side pre-transposes x and all weights so no on-device transposes are
needed, and folds b_lepe through w_proj into an effective bias.
"""

import numpy as np
import ml_dtypes

B, Hs, Ws, C = 8, 32, 32, 512
N = Hs * Ws          # 1024 tokens
HEADS = 16
HD = C // HEADS      # 32
KS = 5
SCALE = float(HD) ** -0.5
NCORES = 8

_BF16 = ml_dtypes.bfloat16

LAST_EXEC_TIME_NS = None
LAST_RESULTS = None


def _build_graph():
    import os as _os
    LOOP = int(_os.environ.get("ATTN_LEPE_LOOP", "1"))
    DBG = _os.environ.get("ATTN_LEPE_DEBUG", "") == "1"
    import concourse.bacc as bacc
    import concourse.mybir as mybir
    import concourse.tile as tile

    dt = mybir.dt
    AF = mybir.ActivationFunctionType

    nc = bacc.Bacc(
        "TRN2",
        target_bir_lowering=False,
        debug=False,
        enable_asserts=False,
        num_devices=NCORES,
    )

    xT_d = nc.dram_tensor("xT", [C, N], dt.bfloat16, kind="ExternalInput")
    wqkT_d = nc.dram_tensor("wqkT", [C, 2 * C], dt.bfloat16, kind="ExternalInput")
    wvT_d = nc.dram_tensor("wvT", [C, C], dt.bfloat16, kind="ExternalInput")
    wpT_d = nc.dram_tensor("wpT", [C, C], dt.bfloat16, kind="ExternalInput")
    ones_d = nc.dram_tensor("ones", [128, 32], dt.bfloat16, kind="ExternalInput")
    # lepe_d[p, (pi*4+g)*128 + q] = (p==q) * w_lepe[128*g+p, inner tap pi]
    lepe_d = nc.dram_tensor("lepe", [128, 9 * 4 * 128], dt.bfloat16,
                            kind="ExternalInput")
    lepec_d = nc.dram_tensor("lepec", [128, KS * KS * 4], dt.float32,
                             kind="ExternalInput")
    beff_d = nc.dram_tensor("beff", [128, 4], dt.float32, kind="ExternalInput")
    out_d = nc.dram_tensor("out", [C, N], dt.float32, kind="ExternalOutput")
    if DBG:
        dbg_pT = nc.dram_tensor("dbg_pT", [128, 2048], dt.bfloat16,
                                kind="ExternalOutput")
        dbg_ao0 = nc.dram_tensor("dbg_ao0", [128, N], dt.bfloat16,
                                 kind="ExternalOutput")

    NT = N // 128   # 8 token tiles
    CT = C // 128   # 4 channel tiles
    NC2 = N // 512  # 2 n-chunks

    taps = [(0, 0)] + [
        (dh, dw) for dh in range(-2, 3) for dw in range(-2, 3) if (dh, dw) != (0, 0)
    ]

    with tile.TileContext(nc) as tc:
        with (
            tc.tile_pool(name="persist", bufs=1) as persist,
            tc.tile_pool(name="pT", bufs=4) as pT_pool,
            tc.tile_pool(name="dr", bufs=3) as dr_pool,
            tc.tile_pool(name="ps_big", bufs=1, space="PSUM") as ps_big,
            tc.tile_pool(name="ps_small", bufs=2, space="PSUM") as ps_small,
            tc.tile_pool(name="ps_den", bufs=1, space="PSUM") as ps_den,
            tc.tile_pool(name="ps_lepe", bufs=1, space="PSUM") as ps_lepe,
        ):
            # ---- persistent SBUF loads ----
            xT = []
            for g in range(CT):
                t = persist.tile([128, N], dt.bfloat16, tag=f"xT{g}", name=f"xT{g}")
                nc.sync.dma_start(t[:], xT_d[g * 128:(g + 1) * 128, :])
                xT.append(t)
            wqkT = []
            for g in range(CT):
                t = persist.tile([128, 2 * C], dt.bfloat16, tag=f"wqkT{g}",
                                 name=f"wqkT{g}")
                nc.sync.dma_start(t[:], wqkT_d[g * 128:(g + 1) * 128, :])
                wqkT.append(t)
            wvT = []
            for g in range(CT):
                t = persist.tile([128, C], dt.bfloat16, tag=f"wvT{g}", name=f"wvT{g}")
                nc.sync.dma_start(t[:], wvT_d[g * 128:(g + 1) * 128, :])
                wvT.append(t)
            ones_sb = persist.tile([128, 32], dt.bfloat16, tag="ones", name="ones_sb")
            nc.sync.dma_start(ones_sb[:], ones_d[:, :])
            # non-critical loads (LePE table, proj weights) are deferred past
            # the head kickoff so the x/w_qk/w_v preload gets full DMA
            # bandwidth
            wpT = [persist.tile([128, C], dt.bfloat16, tag=f"wpT{g}",
                                name=f"wpT{g}") for g in range(CT)]
            lepe_w = persist.tile([128, 9 * 4 * 128], dt.bfloat16, tag="lepe",
                                  name="lepe_w")
            beff_sb = persist.tile([128, 4], dt.float32, tag="beff", name="beff_sb")
            lepec_sb = persist.tile([128, KS * KS * 4], dt.float32, tag="lepec",
                                    name="lepec_sb")

            def load_noncritical():
                nc.sync.dma_start(lepe_w[:], lepe_d[:, :])
                nc.sync.dma_start(lepec_sb[:], lepec_d[:, :])
                for g in range(CT):
                    nc.sync.dma_start(wpT[g][:], wpT_d[g * 128:(g + 1) * 128, :])
                nc.sync.dma_start(beff_sb[:], beff_d[:, :])

            for _it in range(LOOP):
                # ---------- tiles ----------
                v_sb = [persist.tile([128, 512], dt.bfloat16, tag=f"v{m}",
                                     name=f"v{m}") for m in range(NT)]
                qk_sb = [
                    persist.tile([128, N], dt.bfloat16, tag=f"qk{f}", name=f"qk{f}")
                    for f in range(8)
                ]
                aoT = [persist.tile([128, N], dt.bfloat16, tag=f"aoT{g}",
                                    name=f"aoT{g}") for g in range(4)]
                yT_sb = [persist.tile([128, N], dt.float32, tag=f"yT{co}",
                                      name=f"yT{co}") for co in range(CT)]
                x3 = [xT[g][:].rearrange("p (h w) -> p h w", w=Ws)
                      for g in range(CT)]

                def emit_v(m):
                    v_ps = ps_small.tile([128, 512], dt.float32, tag="sm",
                                         name=f"vps{m}")
                    for c in range(CT):
                        nc.tensor.matmul(
                            out=v_ps[:],
                            lhsT=xT[c][:, m * 128:(m + 1) * 128],
                            rhs=wvT[c][:],
                            start=(c == 0), stop=(c == CT - 1),
                        )
                    nc.vector.tensor_copy(v_sb[m][:], v_ps[:])

                def emit_qk(f, nc2):
                    qk_ps = ps_small.tile([128, 512], dt.float32, tag="sm",
                                          name=f"qkps{f}_{nc2}")
                    for c in range(CT):
                        nc.tensor.matmul(
                            out=qk_ps[:],
                            lhsT=wqkT[c][:, f * 128:(f + 1) * 128],
                            rhs=xT[c][:, nc2 * 512:(nc2 + 1) * 512],
                            start=(c == 0), stop=(c == CT - 1),
                        )
                    nc.vector.tensor_copy(
                        qk_sb[f][:, nc2 * 512:(nc2 + 1) * 512], qk_ps[:])

                def emit_proj(co, nc2):
                    ncs = slice(nc2 * 512, (nc2 + 1) * 512)
                    y_ps = ps_small.tile([128, 512], dt.float32, tag="sm",
                                         name=f"yps{co}_{nc2}")
                    for c in range(CT):
                        nc.tensor.matmul(
                            out=y_ps[:],
                            lhsT=wpT[c][:, co * 128:(co + 1) * 128],
                            rhs=aoT[c][:, ncs],
                            start=(c == 0), stop=(c == CT - 1),
                        )
                    nc.vector.tensor_scalar_add(
                        yT_sb[co][:, ncs], y_ps[:], beff_sb[:, co:co + 1])
                    # ship each output half as soon as its proj is done
                    nc.sync.dma_start(out_d[co * 128:(co + 1) * 128, ncs],
                                      yT_sb[co][:, ncs])

                # 3x3 inner taps on TensorE (diag matmuls), 16 outer-ring
                # taps on VectorE (fused affine MAC)
                pe_taps = [(dh, dw) for (dh, dw) in taps
                           if abs(dh) <= 1 and abs(dw) <= 1]
                dve_taps = [t for t in taps if t not in pe_taps]

                def lepe_mms(g, hb, lp3):
                    mms = []
                    for pi, (dh, dw) in enumerate(pe_taps):
                        r0, r1 = max(0, -dh), Hs - max(0, dh)
                        w0, w1 = max(0, -dw), Ws - max(0, dw)
                        hr0, hr1 = max(r0, hb * 16), min(r1, hb * 16 + 16)
                        if hr0 >= hr1:
                            continue
                        diag = lepe_w[:, (pi * 4 + g) * 128:(pi * 4 + g + 1) * 128]

                        def mm(pi=pi, hr0=hr0, hr1=hr1, w0=w0, w1=w1,
                               dh=dh, dw=dw, diag=diag, lp3=lp3, g=g, hb=hb):
                            nc.tensor.matmul(
                                out=lp3[:, hr0 - hb * 16:hr1 - hb * 16, w0:w1],
                                lhsT=diag,
                                rhs=x3[g][:, hr0 + dh:hr1 + dh, w0 + dw:w1 + dw],
                                start=(pi == 0), stop=(pi == len(pe_taps) - 1),
                                skip_group_check=True,
                            )
                        mms.append(mm)
                    return mms

                def lepe_dve_units(g, hb, acc):
                    acc3 = acc[:].rearrange("p (h w) -> p h w", w=Ws)
                    units = []
                    for dh, dw in dve_taps:
                        ti = taps.index((dh, dw))
                        r0, r1 = max(0, -dh), Hs - max(0, dh)
                        w0, w1 = max(0, -dw), Ws - max(0, dw)
                        hr0, hr1 = max(r0, hb * 16), min(r1, hb * 16 + 16)
                        if hr0 >= hr1:
                            continue

                        def u(ti=ti, hr0=hr0, hr1=hr1, w0=w0, w1=w1,
                              dh=dh, dw=dw, acc3=acc3, g=g, hb=hb):
                            dst = acc3[:, hr0 - hb * 16:hr1 - hb * 16, w0:w1]
                            nc.vector.affine_then_add(
                                out=dst,
                                in0=x3[g][:, hr0 + dh:hr1 + dh, w0 + dw:w1 + dw],
                                in1=dst,
                                scale=lepec_sb[:, ti * 4 + g:ti * 4 + g + 1],
                                bias=0.0,
                            )
                        units.append(u)
                    return units

                # ---------- head: minimum to start sweep (nc2=0, g=0) ----------
                emit_qk(4, 0)
                emit_qk(0, 0)
                if _it == 0:
                    load_noncritical()

                # filler units with emission deadlines (global step index)
                fillers = [(1, lambda: emit_v(0)), (2, lambda: emit_v(1))]
                for m in range(2, NT):
                    fillers.append((m + 1, lambda m=m: emit_v(m)))
                qk_sched = [((4, 1), 3), ((5, 0), 5), ((5, 1), 6), ((1, 0), 7),
                            ((6, 0), 13), ((6, 1), 14), ((2, 0), 15),
                            ((7, 0), 21), ((7, 1), 22), ((3, 0), 23),
                            ((0, 1), 30), ((1, 1), 38), ((2, 1), 46),
                            ((3, 1), 54)]
                for (f, nc2), dl in qk_sched:
                    fillers.append((dl, lambda f=f, nc2=nc2: emit_qk(f, nc2)))
                fillers.sort(key=lambda x: x[0])
                late = [(co, 0) for co in range(CT)]  # proj nc0 halves

                # ---------- 64-step flat pipeline over sweeps (nc2, g) ----------
                sweeps = [(nc2, g) for nc2 in range(NC2) for g in range(4)]
                steps = [(nc2, g, m) for (nc2, g) in sweeps for m in range(NT)]
                state = {}

                def sweep_tiles(nc2, g):
                    out_ps = ps_small.tile([128, 512], dt.float32, tag="sm",
                                           name=f"outps{g}_{nc2}")
                    den_ps = ps_den.tile([128, 512], dt.float32, tag="den",
                                         name=f"denps{g}_{nc2}")
                    lp_ps = ps_lepe.tile([128, 512], dt.float32, tag="lp",
                                         name=f"lp{g}_{nc2}")
                    lp3 = lp_ps[:].rearrange("p (h w) -> p h w", w=Ws)
                    acc = dr_pool.tile([128, 512], dt.bfloat16, tag="dveacc",
                                       name=f"acc{g}_{nc2}")
                    nc.gpsimd.memset(acc[:], 0.0)
                    return dict(out_ps=out_ps, den_ps=den_ps, lp_ps=lp_ps,
                                acc=acc, lepe=lepe_mms(g, nc2, lp3),
                                dve=lepe_dve_units(g, nc2, acc))

                def emit_sT(nc2, g, m):
                    ncs = slice(nc2 * 512, (nc2 + 1) * 512)
                    q_t, k_t = qk_sb[g], qk_sb[4 + g]
                    sT_ps = ps_big.tile([128, 2048], dt.float32, tag="big",
                                        name=f"sT{g}_{nc2}_{m}")
                    for j in range(4):
                        nc.tensor.matmul(
                            out=sT_ps[:, j * 512:(j + 1) * 512],
                            lhsT=k_t[j * 32:(j + 1) * 32, m * 128:(m + 1) * 128],
                            rhs=q_t[j * 32:(j + 1) * 32, ncs],
                            start=True, stop=True,
                            tile_position=(j * 32, 0),
                        )
                    pT = pT_pool.tile([128, 2048], dt.bfloat16, tag="pT",
                                      name=f"pT{g}_{nc2}_{m}")
                    nc.scalar.activation(pT[:], sT_ps[:], AF.Exp, scale=SCALE)
                    if DBG and _it == 0 and g == 0 and nc2 == 0 and m == 0:
                        nc.sync.dma_start(dbg_pT[:, :], pT[:])
                    return pT

                def emit_pv(nc2, g, m, pT):
                    st = state[(nc2, g)]
                    for j in range(4):
                        h = 4 * g + j
                        nc.tensor.matmul(
                            out=st["out_ps"][j * 32:(j + 1) * 32, :],
                            lhsT=v_sb[m][:, h * 32:(h + 1) * 32],
                            rhs=pT[:, j * 512:(j + 1) * 512],
                            start=(m == 0), stop=(m == NT - 1),
                            tile_position=(0, j * 32),
                            skip_group_check=True,
                        )
                        nc.tensor.matmul(
                            out=st["den_ps"][j * 32:(j + 1) * 32, :],
                            lhsT=ones_sb[:, 0:32],
                            rhs=pT[:, j * 512:(j + 1) * 512],
                            start=(m == 0), stop=(m == NT - 1),
                            tile_position=(0, j * 32),
                            skip_group_check=True,
                        )

                def emit_epilogue(nc2, g):
                    st = state.pop((nc2, g))
                    ncs = slice(nc2 * 512, (nc2 + 1) * 512)
                    drec = dr_pool.tile([128, 512], dt.float32, tag="drec",
                                        name="drec")
                    nc.vector.reciprocal_approx_fast(out=drec[:],
                                                     in_=st["den_ps"][:])
                    tmp = dr_pool.tile([128, 512], dt.float32, tag="ntmp",
                                       name="ntmp")
                    nc.vector.tensor_mul(tmp[:], st["out_ps"][:], drec[:])
                    nc.vector.tensor_add(
                        aoT[g][:, ncs], tmp[:], aoT[g][:, ncs])
                    if DBG and _it == 0 and g == 0 and nc2 == 1:
                        nc.sync.dma_start(dbg_ao0[:, :], aoT[g][:])

                prev = None       # (nc2, g, m, pT)
                for i, (nc2, g, m) in enumerate(steps):
                    while fillers and fillers[0][0] <= i:
                        fillers.pop(0)[1]()
                    if m == 0:
                        state[(nc2, g)] = sweep_tiles(nc2, g)
                    pT = emit_sT(nc2, g, m)
                    if prev is not None:
                        pnc2, pg, pm, ppT = prev
                        emit_pv(pnc2, pg, pm, ppT)
                        if pm == NT - 1:
                            emit_epilogue(pnc2, pg)
                    st = state[(nc2, g)]
                    # hold the sweep's first LePE ops one step so they don't
                    # stall on the previous sweep's epilogue chain
                    nmm = 0 if m == 0 else (1 if m < NT - 1 else len(st["lepe"]))
                    for _ in range(min(nmm, len(st["lepe"]))):
                        st["lepe"].pop(0)()
                    ndve = 0 if m == 0 else (3 if m < NT - 1 else len(st["dve"]))
                    for _ in range(min(ndve, len(st["dve"]))):
                        st["dve"].pop(0)()
                    if m == NT - 1:
                        # pre-merge LePE (PE psum + DVE acc) into aoT now,
                        # independent of exp(m)/p@v(m); the post-p@v epilogue
                        # then only needs recip -> mul -> one add
                        ncs_s = slice(nc2 * 512, (nc2 + 1) * 512)
                        nc.vector.tensor_add(
                            aoT[g][:, ncs_s], st["lp_ps"][:], st["acc"][:])
                    if late and i >= 34:
                        emit_proj(*late.pop(0))
                    elif fillers and (i % 2 == 1 or i < 8):
                        fillers.pop(0)[1]()
                    prev = (nc2, g, m, pT)

                # tail
                pnc2, pg, pm, ppT = prev
                emit_pv(pnc2, pg, pm, ppT)
                emit_epilogue(pnc2, pg)
                while late:
                    emit_proj(*late.pop(0))
                for co in range(CT):
                    emit_proj(co, 1)

    nc.finalize()
    return nc


_GRAPH = None


def kernel(x, w_qkv, w_proj, b_proj, w_lepe, b_lepe, _trace=False):
    global _GRAPH, LAST_EXEC_TIME_NS, LAST_RESULTS
    from concourse.bass_utils import run_bass_kernel_spmd

    x = np.asarray(x, dtype=np.float32)
    w_qkv = np.asarray(w_qkv, dtype=np.float32)
    w_proj = np.asarray(w_proj, dtype=np.float32)
    b_proj = np.asarray(b_proj, dtype=np.float32)
    w_lepe = np.asarray(w_lepe, dtype=np.float32)
    b_lepe = np.asarray(b_lepe, dtype=np.float32)

    wqkT = np.ascontiguousarray(w_qkv[:2 * C, :].T).astype(_BF16)   # [C, 2C]
    wvT = np.ascontiguousarray(w_qkv[2 * C:, :].T).astype(_BF16)    # [C, C]
    wpT = np.ascontiguousarray(w_proj.T).astype(_BF16)              # [C, C]
    beff = (w_proj @ b_lepe + b_proj).astype(np.float32)            # [C]
    beff_t = np.ascontiguousarray(beff.reshape(4, 128).T)           # [128, 4]

    taps = [(0, 0)] + [
        (dh, dw) for dh in range(-2, 3) for dw in range(-2, 3) if (dh, dw) != (0, 0)
    ]
    wl = w_lepe.reshape(C, KS, KS)  # tap (dh,dw) -> kernel[dh+2, dw+2]
    pe_taps = [(dh, dw) for (dh, dw) in taps if abs(dh) <= 1 and abs(dw) <= 1]
    lepe_flat = np.zeros((128, 9 * 4 * 128), dtype=_BF16)
    for pi, (dh, dw) in enumerate(pe_taps):
        for g in range(4):
            col0 = (pi * 4 + g) * 128
            wcol = wl[g * 128:(g + 1) * 128, dh + 2, dw + 2].astype(_BF16)
            lepe_flat[np.arange(128), col0 + np.arange(128)] = wcol
    ones128 = np.ones((128, 32), dtype=_BF16)
    lepe_col = np.zeros((128, KS * KS * 4), dtype=np.float32)
    for ti, (dh, dw) in enumerate(taps):
        for g in range(4):
            lepe_col[:, ti * 4 + g] = wl[g * 128:(g + 1) * 128, dh + 2, dw + 2]

    in_maps = []
    for b in range(NCORES):
        xT = np.ascontiguousarray(x[b].reshape(N, C).T).astype(_BF16)  # [C, N]
        in_maps.append({
            "xT": xT,
            "wqkT": wqkT,
            "wvT": wvT,
            "wpT": wpT,
            "ones": ones128,
            "lepe": lepe_flat,
            "lepec": lepe_col,
            "beff": beff_t,
        })

    if _GRAPH is None:
        _GRAPH = _build_graph()

    res = run_bass_kernel_spmd(_GRAPH, in_maps, list(range(NCORES)), trace=_trace)
    LAST_EXEC_TIME_NS = res.exec_time_ns
    LAST_RESULTS = res

    out = np.empty((B, Hs, Ws, C), dtype=np.float32)
    for b in range(NCORES):
        yT = np.asarray(res.results[b]["out"], dtype=np.float32)  # [C, N]
        out[b] = yT.T.reshape(Hs, Ws, C)
    return out


# revision 33
# speedup vs baseline: 1.0142x; 1.0049x over previous
"""AttentionLePE distributed Trainium2 kernel.

Strategy: pure data-parallel over batch (8 batch elements -> 8 NeuronCores,
no collectives). Per core, the full attention block runs with:
  - bf16 matmuls on TensorE (qkv, s^T = k q^T with 4-head row-packing,
    p@v + all-ones denominator matmuls with 4-head col-packing, proj)
  - softmax exp on ScalarE straight out of PSUM (no max-subtraction: logits
    are ~N(0,1) so exp is safe in f32)
  - LePE 5x5 depthwise conv split across engines: the 3x3 inner taps as
    accumulating diagonal matmuls on TensorE with spatially shifted access
    patterns (zero-pad handled by trimming), the 16 outer-ring taps as fused
    affine MACs (affine_then_add) on VectorE
  - normalization via all-ones matmul row sums (replicated to each head's 32
    output rows) -> fast reciprocal on VectorE straight from PSUM -> fused
    multiply during PSUM evacuation

The whole kernel is software-pipelined at emission time as one flat 64-step
sweep sequence: each engine's in-order stream gets p@v of step m-1 plus
deadline-scheduled filler (v/qk tiles, LePE taps, early proj halves) between
s^T(m) and s^T(m+1), so TensorE stays busy while ScalarE runs exp(m) and
exp never stalls at sweep boundaries.

Host side pre-transposes x and all weights so no on-device transposes are
needed, and folds b_lepe through w_proj into an effective bias.
"""

import numpy as np
import ml_dtypes

B, Hs, Ws, C = 8, 32, 32, 512
N = Hs * Ws          # 1024 tokens
HEADS = 16
HD = C // HEADS      # 32
KS = 5
SCALE = float(HD) ** -0.5
NCORES = 8

_BF16 = ml_dtypes.bfloat16

LAST_EXEC_TIME_NS = None
LAST_RESULTS = None


def _build_graph():
    import os as _os
    LOOP = int(_os.environ.get("ATTN_LEPE_LOOP", "1"))
    DBG = _os.environ.get("ATTN_LEPE_DEBUG", "") == "1"
    import concourse.bacc as bacc
    import concourse.mybir as mybir
    import concourse.tile as tile

    dt = mybir.dt
    AF = mybir.ActivationFunctionType

    nc = bacc.Bacc(
        "TRN2",
        target_bir_lowering=False,
        debug=False,
        enable_asserts=False,
        num_devices=NCORES,
    )

    xT_d = nc.dram_tensor("xT", [C, N], dt.bfloat16, kind="ExternalInput")
    wqkT_d = nc.dram_tensor("wqkT", [C, 2 * C], dt.bfloat16, kind="ExternalInput")
    wvT_d = nc.dram_tensor("wvT", [C, C], dt.bfloat16, kind="ExternalInput")
    wpT_d = nc.dram_tensor("wpT", [C, C], dt.bfloat16, kind="ExternalInput")
    ones_d = nc.dram_tensor("ones", [128, 32], dt.bfloat16, kind="ExternalInput")
    # lepe_d[p, (pi*4+g)*128 + q] = (p==q) * w_lepe[128*g+p, inner tap pi]
    lepe_d = nc.dram_tensor("lepe", [128, 9 * 4 * 128], dt.bfloat16,
                            kind="ExternalInput")
    lepec_d = nc.dram_tensor("lepec", [128, KS * KS * 4], dt.float32,
                             kind="ExternalInput")
    beff_d = nc.dram_tensor("beff", [128, 4], dt.float32, kind="ExternalInput")
    out_d = nc.dram_tensor("out", [C, N], dt.float32, kind="ExternalOutput")
    if DBG:
        dbg_pT = nc.dram_tensor("dbg_pT", [128, 2048], dt.bfloat16,
                                kind="ExternalOutput")
        dbg_ao0 = nc.dram_tensor("dbg_ao0", [128, N], dt.bfloat16,
                                 kind="ExternalOutput")

    NT = N // 128   # 8 token tiles
    CT = C // 128   # 4 channel tiles
    NC2 = N // 512  # 2 n-chunks

    taps = [(0, 0)] + [
        (dh, dw) for dh in range(-2, 3) for dw in range(-2, 3) if (dh, dw) != (0, 0)
    ]

    with tile.TileContext(nc) as tc:
        with (
            tc.tile_pool(name="persist", bufs=1) as persist,
            tc.tile_pool(name="pT", bufs=4) as pT_pool,
            tc.tile_pool(name="dr", bufs=3) as dr_pool,
            tc.tile_pool(name="ps_big", bufs=1, space="PSUM") as ps_big,
            tc.tile_pool(name="ps_small", bufs=2, space="PSUM") as ps_small,
            tc.tile_pool(name="ps_den", bufs=1, space="PSUM") as ps_den,
            tc.tile_pool(name="ps_lepe", bufs=1, space="PSUM") as ps_lepe,
        ):
            # ---- persistent SBUF loads ----
            xT = []
            for g in range(CT):
                t = persist.tile([128, N], dt.bfloat16, tag=f"xT{g}", name=f"xT{g}")
                nc.sync.dma_start(t[:], xT_d[g * 128:(g + 1) * 128, :])
                xT.append(t)
            wqkT = []
            for g in range(CT):
                t = persist.tile([128, 2 * C], dt.bfloat16, tag=f"wqkT{g}",
                                 name=f"wqkT{g}")
                nc.sync.dma_start(t[:], wqkT_d[g * 128:(g + 1) * 128, :])
                wqkT.append(t)
            wvT = []
            for g in range(CT):
                t = persist.tile([128, C], dt.bfloat16, tag=f"wvT{g}", name=f"wvT{g}")
                nc.sync.dma_start(t[:], wvT_d[g * 128:(g + 1) * 128, :])
                wvT.append(t)
            ones_sb = persist.tile([128, 32], dt.bfloat16, tag="ones", name="ones_sb")
            nc.sync.dma_start(ones_sb[:], ones_d[:, :])
            # non-critical loads (LePE table, proj weights) are deferred past
            # the head kickoff so the x/w_qk/w_v preload gets full DMA
            # bandwidth
            wpT = [persist.tile([128, C], dt.bfloat16, tag=f"wpT{g}",
                                name=f"wpT{g}") for g in range(CT)]
            lepe_w = persist.tile([128, 9 * 4 * 128], dt.bfloat16, tag="lepe",
                                  name="lepe_w")
            beff_sb = persist.tile([128, 4], dt.float32, tag="beff", name="beff_sb")
            lepec_sb = persist.tile([128, KS * KS * 4], dt.float32, tag="lepec",
                                    name="lepec_sb")

            def load_noncritical():
                nc.sync.dma_start(lepe_w[:], lepe_d[:, :])
                nc.sync.dma_start(lepec_sb[:], lepec_d[:, :])
                for g in range(CT):
                    nc.sync.dma_start(wpT[g][:], wpT_d[g * 128:(g + 1) * 128, :])
                nc.sync.dma_start(beff_sb[:], beff_d[:, :])

            for _it in range(LOOP):
                # ---------- tiles ----------
                v_sb = [persist.tile([128, 512], dt.bfloat16, tag=f"v{m}",
                                     name=f"v{m}") for m in range(NT)]
                qk_sb = [
                    persist.tile([128, N], dt.bfloat16, tag=f"qk{f}", name=f"qk{f}")
                    for f in range(8)
                ]
                aoT = [persist.tile([128, N], dt.bfloat16, tag=f"aoT{g}",
                                    name=f"aoT{g}") for g in range(4)]
                yT_sb = [persist.tile([128, N], dt.float32, tag=f"yT{co}",
                                      name=f"yT{co}") for co in range(CT)]
                x3 = [xT[g][:].rearrange("p (h w) -> p h w", w=Ws)
                      for g in range(CT)]

                def emit_v(m):
                    v_ps = ps_small.tile([128, 512], dt.float32, tag="sm",
                                         name=f"vps{m}")
                    for c in range(CT):
                        nc.tensor.matmul(
                            out=v_ps[:],
                            lhsT=xT[c][:, m * 128:(m + 1) * 128],
                            rhs=wvT[c][:],
                            start=(c == 0), stop=(c == CT - 1),
                        )
                    nc.vector.tensor_copy(v_sb[m][:], v_ps[:])

                def emit_qk(f, nc2):
                    qk_ps = ps_small.tile([128, 512], dt.float32, tag="sm",
                                          name=f"qkps{f}_{nc2}")
                    for c in range(CT):
                        nc.tensor.matmul(
                            out=qk_ps[:],
                            lhsT=wqkT[c][:, f * 128:(f + 1) * 128],
                            rhs=xT[c][:, nc2 * 512:(nc2 + 1) * 512],
                            start=(c == 0), stop=(c == CT - 1),
                        )
                    nc.vector.tensor_copy(
                        qk_sb[f][:, nc2 * 512:(nc2 + 1) * 512], qk_ps[:])

                def emit_proj(co, nc2):
                    ncs = slice(nc2 * 512, (nc2 + 1) * 512)
                    y_ps = ps_small.tile([128, 512], dt.float32, tag="sm",
                                         name=f"yps{co}_{nc2}")
                    for c in range(CT):
                        nc.tensor.matmul(
                            out=y_ps[:],
                            lhsT=wpT[c][:, co * 128:(co + 1) * 128],
                            rhs=aoT[c][:, ncs],
                            start=(c == 0), stop=(c == CT - 1),
                        )
                    nc.vector.tensor_scalar_add(
                        yT_sb[co][:, ncs], y_ps[:], beff_sb[:, co:co + 1])
                    # ship each output half as soon as its proj is done
                    nc.sync.dma_start(out_d[co * 128:(co + 1) * 128, ncs],
                                      yT_sb[co][:, ncs])

                # 3x3 inner taps on TensorE (diag matmuls), 16 outer-ring
                # taps on VectorE (fused affine MAC)
                pe_taps = [(dh, dw) for (dh, dw) in taps
                           if abs(dh) <= 1 and abs(dw) <= 1]
                dve_taps = [t for t in taps if t not in pe_taps]

                def lepe_mms(g, hb, lp3):
                    mms = []
                    for pi, (dh, dw) in enumerate(pe_taps):
                        r0, r1 = max(0, -dh), Hs - max(0, dh)
                        w0, w1 = max(0, -dw), Ws - max(0, dw)
                        hr0, hr1 = max(r0, hb * 16), min(r1, hb * 16 + 16)
                        if hr0 >= hr1:
                            continue
                        diag = lepe_w[:, (pi * 4 + g) * 128:(pi * 4 + g + 1) * 128]

                        def mm(pi=pi, hr0=hr0, hr1=hr1, w0=w0, w1=w1,
                               dh=dh, dw=dw, diag=diag, lp3=lp3, g=g, hb=hb):
                            nc.tensor.matmul(
                                out=lp3[:, hr0 - hb * 16:hr1 - hb * 16, w0:w1],
                                lhsT=diag,
                                rhs=x3[g][:, hr0 + dh:hr1 + dh, w0 + dw:w1 + dw],
                                start=(pi == 0), stop=(pi == len(pe_taps) - 1),
                                skip_group_check=True,
                            )
                        mms.append(mm)
                    return mms

                def lepe_dve_units(g, hb, acc):
                    acc3 = acc[:].rearrange("p (h w) -> p h w", w=Ws)
                    units = []
                    for dh, dw in dve_taps:
                        ti = taps.index((dh, dw))
                        r0, r1 = max(0, -dh), Hs - max(0, dh)
                        w0, w1 = max(0, -dw), Ws - max(0, dw)
                        hr0, hr1 = max(r0, hb * 16), min(r1, hb * 16 + 16)
                        if hr0 >= hr1:
                            continue

                        def u(ti=ti, hr0=hr0, hr1=hr1, w0=w0, w1=w1,
                              dh=dh, dw=dw, acc3=acc3, g=g, hb=hb):
                            dst = acc3[:, hr0 - hb * 16:hr1 - hb * 16, w0:w1]
                            nc.vector.affine_then_add(
                                out=dst,
                                in0=x3[g][:, hr0 + dh:hr1 + dh, w0 + dw:w1 + dw],
                                in1=dst,
                                scale=lepec_sb[:, ti * 4 + g:ti * 4 + g + 1],
                                bias=0.0,
                            )
                        units.append(u)
                    return units

                # ---------- head: minimum to start sweep (nc2=0, g=0) ----------
                emit_qk(4, 0)
                emit_qk(0, 0)
                if _it == 0:
                    load_noncritical()

                # filler units with emission deadlines (global step index)
                fillers = [(1, lambda: emit_v(0)), (2, lambda: emit_v(1))]
                for m in range(2, NT):
                    fillers.append((m + 1, lambda m=m: emit_v(m)))
                qk_sched = [((4, 1), 3), ((5, 0), 5), ((5, 1), 6), ((1, 0), 7),
                            ((6, 0), 13), ((6, 1), 14), ((2, 0), 15),
                            ((7, 0), 21), ((7, 1), 22), ((3, 0), 23),
                            ((0, 1), 30), ((1, 1), 38), ((2, 1), 46),
                            ((3, 1), 54)]
                for (f, nc2), dl in qk_sched:
                    fillers.append((dl, lambda f=f, nc2=nc2: emit_qk(f, nc2)))
                fillers.sort(key=lambda x: x[0])
                late = [(co, 0) for co in range(CT)]  # proj nc0 halves

                # ---------- 64-step flat pipeline over sweeps (nc2, g) ----------
                sweeps = [(nc2, g) for nc2 in range(NC2) for g in range(4)]
                steps = [(nc2, g, m) for (nc2, g) in sweeps for m in range(NT)]
                state = {}

                def sweep_tiles(nc2, g):
                    out_ps = ps_small.tile([128, 512], dt.float32, tag="sm",
                                           name=f"outps{g}_{nc2}")
                    den_ps = ps_den.tile([128, 512], dt.float32, tag="den",
                                         name=f"denps{g}_{nc2}")
                    lp_ps = ps_lepe.tile([128, 512], dt.float32, tag="lp",
                                         name=f"lp{g}_{nc2}")
                    lp3 = lp_ps[:].rearrange("p (h w) -> p h w", w=Ws)
                    acc = dr_pool.tile([128, 512], dt.bfloat16, tag="dveacc",
                                       name=f"acc{g}_{nc2}")
                    nc.gpsimd.memset(acc[:], 0.0)
                    return dict(out_ps=out_ps, den_ps=den_ps, lp_ps=lp_ps,
                                acc=acc, lepe=lepe_mms(g, nc2, lp3),
                                dve=lepe_dve_units(g, nc2, acc))

                def emit_sT(nc2, g, m):
                    ncs = slice(nc2 * 512, (nc2 + 1) * 512)
                    q_t, k_t = qk_sb[g], qk_sb[4 + g]
                    sT_ps = ps_big.tile([128, 2048], dt.float32, tag="big",
                                        name=f"sT{g}_{nc2}_{m}")
                    for j in range(4):
                        nc.tensor.matmul(
                            out=sT_ps[:, j * 512:(j + 1) * 512],
                            lhsT=k_t[j * 32:(j + 1) * 32, m * 128:(m + 1) * 128],
                            rhs=q_t[j * 32:(j + 1) * 32, ncs],
                            start=True, stop=True,
                            tile_position=(j * 32, 0),
                        )
                    pT = pT_pool.tile([128, 2048], dt.bfloat16, tag="pT",
                                      name=f"pT{g}_{nc2}_{m}")
                    nc.scalar.activation(pT[:], sT_ps[:], AF.Exp, scale=SCALE)
                    if DBG and _it == 0 and g == 0 and nc2 == 0 and m == 0:
                        nc.sync.dma_start(dbg_pT[:, :], pT[:])
                    return pT

                def emit_pv(nc2, g, m, pT):
                    st = state[(nc2, g)]
                    for j in range(4):
                        h = 4 * g + j
                        nc.tensor.matmul(
                            out=st["out_ps"][j * 32:(j + 1) * 32, :],
                            lhsT=v_sb[m][:, h * 32:(h + 1) * 32],
                            rhs=pT[:, j * 512:(j + 1) * 512],
                            start=(m == 0), stop=(m == NT - 1),
                            tile_position=(0, j * 32),
                            skip_group_check=True,
                        )
                        nc.tensor.matmul(
                            out=st["den_ps"][j * 32:(j + 1) * 32, :],
                            lhsT=ones_sb[:, 0:32],
                            rhs=pT[:, j * 512:(j + 1) * 512],
                            start=(m == 0), stop=(m == NT - 1),
                            tile_position=(0, j * 32),
                            skip_group_check=True,
                        )

                def emit_epilogue(nc2, g):
                    st = state.pop((nc2, g))
                    ncs = slice(nc2 * 512, (nc2 + 1) * 512)
                    drec = dr_pool.tile([128, 512], dt.float32, tag="drec",
                                        name="drec")
                    nc.vector.reciprocal_approx_fast(out=drec[:],
                                                     in_=st["den_ps"][:])
                    tmp = dr_pool.tile([128, 512], dt.float32, tag="ntmp",
                                       name="ntmp")
                    nc.vector.tensor_mul(tmp[:], st["out_ps"][:], drec[:])
                    nc.vector.tensor_add(
                        aoT[g][:, ncs], tmp[:], aoT[g][:, ncs])
                    if DBG and _it == 0 and g == 0 and nc2 == 1:
                        nc.sync.dma_start(dbg_ao0[:, :], aoT[g][:])

                prev = None       # (nc2, g, m, pT)
                for i, (nc2, g, m) in enumerate(steps):
                    while fillers and fillers[0][0] <= i:
                        fillers.pop(0)[1]()
                    if m == 0:
                        state[(nc2, g)] = sweep_tiles(nc2, g)
                    pT = emit_sT(nc2, g, m)
                    if prev is not None:
                        pnc2, pg, pm, ppT = prev
                        emit_pv(pnc2, pg, pm, ppT)
                        if pm == NT - 1:
                            emit_epilogue(pnc2, pg)
                    st = state[(nc2, g)]
                    # hold the sweep's first LePE ops one step so they don't
                    # stall on the previous sweep's epilogue chain
                    nmm = 0 if m == 0 else (1 if m < NT - 1 else len(st["lepe"]))
                    for _ in range(min(nmm, len(st["lepe"]))):
                        st["lepe"].pop(0)()
                    ndve = 0 if m == 0 else (3 if m < NT - 1 else len(st["dve"]))
                    for _ in range(min(ndve, len(st["dve"]))):
                        st["dve"].pop(0)()
                    if m == NT - 1:
                        # pre-merge LePE (PE psum + DVE acc) into aoT now,
                        # independent of exp(m)/p@v(m); the post-p@v epilogue
                        # then only needs recip -> mul -> one add
                        ncs_s = slice(nc2 * 512, (nc2 + 1) * 512)
                        nc.vector.tensor_add(
                            aoT[g][:, ncs_s], st["lp_ps"][:], st["acc"][:])
                    if late and i >= 40 and i % 5 == 0:
                        emit_proj(*late.pop(0))
                    elif fillers and (i % 2 == 1 or i < 8):
                        fillers.pop(0)[1]()
                    prev = (nc2, g, m, pT)

                # tail
                pnc2, pg, pm, ppT = prev
                emit_pv(pnc2, pg, pm, ppT)
                emit_epilogue(pnc2, pg)
                while late:
                    emit_proj(*late.pop(0))
                for co in range(CT):
                    emit_proj(co, 1)

    nc.finalize()
    return nc


_GRAPH = None


def kernel(x, w_qkv, w_proj, b_proj, w_lepe, b_lepe, _trace=False):
    global _GRAPH, LAST_EXEC_TIME_NS, LAST_RESULTS
    from concourse.bass_utils import run_bass_kernel_spmd

    x = np.asarray(x, dtype=np.float32)
    w_qkv = np.asarray(w_qkv, dtype=np.float32)
    w_proj = np.asarray(w_proj, dtype=np.float32)
    b_proj = np.asarray(b_proj, dtype=np.float32)
    w_lepe = np.asarray(w_lepe, dtype=np.float32)
    b_lepe = np.asarray(b_lepe, dtype=np.float32)

    wqkT = np.ascontiguousarray(w_qkv[:2 * C, :].T).astype(_BF16)   # [C, 2C]
    wvT = np.ascontiguousarray(w_qkv[2 * C:, :].T).astype(_BF16)    # [C, C]
    wpT = np.ascontiguousarray(w_proj.T).astype(_BF16)              # [C, C]
    beff = (w_proj @ b_lepe + b_proj).astype(np.float32)            # [C]
    beff_t = np.ascontiguousarray(beff.reshape(4, 128).T)           # [128, 4]

    taps = [(0, 0)] + [
        (dh, dw) for dh in range(-2, 3) for dw in range(-2, 3) if (dh, dw) != (0, 0)
    ]
    wl = w_lepe.reshape(C, KS, KS)  # tap (dh,dw) -> kernel[dh+2, dw+2]
    pe_taps = [(dh, dw) for (dh, dw) in taps if abs(dh) <= 1 and abs(dw) <= 1]
    lepe_flat = np.zeros((128, 9 * 4 * 128), dtype=_BF16)
    for pi, (dh, dw) in enumerate(pe_taps):
        for g in range(4):
            col0 = (pi * 4 + g) * 128
            wcol = wl[g * 128:(g + 1) * 128, dh + 2, dw + 2].astype(_BF16)
            lepe_flat[np.arange(128), col0 + np.arange(128)] = wcol
    ones128 = np.ones((128, 32), dtype=_BF16)
    lepe_col = np.zeros((128, KS * KS * 4), dtype=np.float32)
    for ti, (dh, dw) in enumerate(taps):
        for g in range(4):
            lepe_col[:, ti * 4 + g] = wl[g * 128:(g + 1) * 128, dh + 2, dw + 2]

    in_maps = []
    for b in range(NCORES):
        xT = np.ascontiguousarray(x[b].reshape(N, C).T).astype(_BF16)  # [C, N]
        in_maps.append({
            "xT": xT,
            "wqkT": wqkT,
            "wvT": wvT,
            "wpT": wpT,
            "ones": ones128,
            "lepe": lepe_flat,
            "lepec": lepe_col,
            "beff": beff_t,
        })

    if _GRAPH is None:
        _GRAPH = _build_graph()

    res = run_bass_kernel_spmd(_GRAPH, in_maps, list(range(NCORES)), trace=_trace)
    LAST_EXEC_TIME_NS = res.exec_time_ns
    LAST_RESULTS = res

    out = np.empty((B, Hs, Ws, C), dtype=np.float32)
    for b in range(NCORES):
        yT = np.asarray(res.results[b]["out"], dtype=np.float32)  # [C, N]
        out[b] = yT.T.reshape(Hs, Ws, C)
    return out
